# revision 1
# baseline (speedup 1.0000x reference)
"""Trainium2 Bass kernel for ContextEncoderModel (siamese LSTM encoder + MLP).

Reference computation (B=256, T=128, E=300, H=512, D=1024):
  lh = LSTM_left (left_embed,  left_lengths)  -> h at t=len-1   [B, H]
  rh = LSTM_right(right_embed, right_lengths) -> h at t=len-1   [B, H]
  x  = relu(concat([lh, rh]) @ trans_w)                          [B, D]
  x  = relu(x @ hidden_ws[0]); x = relu(x @ hidden_ws[1])        [B, D]

Sharding (8 cores): core i handles side s=i//4 (0=left, 1=right) and batch
shard q=i%4 (rows q*64:(q+1)*64).  The LSTM state is kept TRANSPOSED on
device: h^T/c^T live as [128 partitions(h-dim chunk), 4(chunk) x 64(batch)]
folded tiles, so the per-step recurrent matmul is
    z^T[gate-chunk m, b] += W[k-chunk, m-chunk].T @ h^T[k-chunk, b]
with the weights as the stationary operand (bf16 -> fast weight load) and no
per-step transposes anywhere.  The input contribution x_t @ W_x (+ bias, via
an appended ones-row) is folded into the same PSUM accumulation group; those
matmuls have no dependence on h so the Tile scheduler runs them ahead,
filling the PE gap while the gate nonlinearities of the previous step run.

Final-state capture: the reference freezes state past t=len-1; running the
recurrence unmasked and latching h at t=len-1 is equivalent.  A host-built
one-hot mask [T, 128, 256] drives one copy_predicated per step into fh.

The MLP head needs concat(lh, rh):  trans_w is split into top/bottom 512
rows; each core computes its partial [D, 64] product and a pairwise
AllReduce (cores q and 4+q hold the same batch shard) produces the sum, then
both cores redundantly run the tiny 2-layer tail.  Host reads cores 0-3.
"""

import numpy as np
import ml_dtypes

import concourse.mybir as mybir
import concourse.tile as tile
from concourse import bacc
from concourse.bass_utils import run_bass_kernel_spmd

BF16 = mybir.dt.bfloat16
F32 = mybir.dt.float32
AF = mybir.ActivationFunctionType

B, T, E, H, D = 256, 128, 300, 512, 1024
NCORES = 8
BC = B // 4          # 64 batch rows per core (4 shards x 2 sides)
P = 128
G = 4 * H            # 2048 gate pre-activations
KH = H // P          # 4 h-dim chunks
KE_FULL = 2          # full 128-row chunks of E
E2 = E - KE_FULL * P  # 44 leftover rows
MG = G // P          # 16 gate-dim chunks
MD = D // P          # 8 D chunks
FORGET_BIAS = 1.0
MCH = 16             # mask chunk: steps per DMA


def _build(t_steps: int = T):
    nc = bacc.Bacc(
        "TRN2", target_bir_lowering=False, debug=False, num_devices=NCORES
    )
    TB = t_steps * BC
    mch = min(MCH, t_steps)

    def din(name, shape):
        return nc.dram_tensor(name, shape, BF16, kind="ExternalInput").ap()

    xt0 = din("xt0", [P, TB])
    xt1 = din("xt1", [P, TB])
    xt2 = din("xt2", [E2 + 1, TB])          # 44 embed rows + ones row
    wh = din("wh", [P, KH * G])             # wh[p, k*G+c] = W[300+k*128+p, c]
    wx01 = din("wx01", [P, 2 * G])
    wx2 = din("wx2", [E2 + 1, G])           # 44 rows + adjusted-bias row
    masks = nc.dram_tensor(
        "masks", [t_steps // mch, P, mch * KH * BC], mybir.dt.uint8,
        kind="ExternalInput",
    ).ap()
    twt = din("twt", [P, KH * D])           # side's 512 rows of trans_w
    hww = din("hww", [P, 2 * MD * D])       # both hidden layers
    out_t = nc.dram_tensor("out_t", [P, MD * BC], F32, kind="ExternalOutput").ap()

    SB = KH * BC  # 256: folded free size of state tiles

    with tile.TileContext(nc) as tc:
        with (
            tc.tile_pool(name="wpool", bufs=1) as wp,
            tc.tile_pool(name="state", bufs=1) as st,
            tc.tile_pool(name="gates", bufs=4) as gp,
            tc.tile_pool(name="mpool", bufs=2) as mp,
            tc.tile_pool(name="psum", bufs=8, space="PSUM") as psum,
            tc.tile_pool(name="dram", bufs=2, space="DRAM") as dp,
        ):
            # ---- resident weights / inputs ----
            XC = 32  # steps per xt DMA chunk
            nxc = max(1, t_steps // XC)
            xcs = min(XC, t_steps)
            wh_sb_k = [
                wp.tile([P, G], BF16, name=f"whk{k}") for k in range(KH)
            ]
            wx01_sb = wp.tile([P, 2 * G], BF16)
            wx2_sb = wp.tile([E2 + 1, G], BF16)
            twt_sb = wp.tile([P, KH * D], BF16)
            hww_sb = wp.tile([P, 2 * MD * D], BF16)
            xt0_c = []
            xt1_c = []
            xt2_c = []
            nc.sync.dma_start(wx01_sb[:], wx01[:])
            nc.sync.dma_start(wx2_sb[:], wx2[:])
            for ci in range(nxc):
                csl = slice(ci * xcs * BC, (ci + 1) * xcs * BC)
                x0 = wp.tile([P, xcs * BC], BF16, name=f"xt0c{ci}")
                x1 = wp.tile([P, xcs * BC], BF16, name=f"xt1c{ci}")
                x2 = wp.tile([E2 + 1, xcs * BC], BF16, name=f"xt2c{ci}")
                nc.sync.dma_start(x0[:], xt0[:, csl])
                if ci == 0:
                    for k in range(KH):
                        nc.sync.dma_start(wh_sb_k[k][:], wh[:, k * G:(k + 1) * G])
                nc.sync.dma_start(x1[:], xt1[:, csl])
                nc.sync.dma_start(x2[:], xt2[:, csl])
                xt0_c.append(x0)
                xt1_c.append(x1)
                xt2_c.append(x2)
            nc.sync.dma_start(twt_sb[:], twt[:])
            nc.sync.dma_start(hww_sb[:], hww[:])

            # ---- persistent state ----
            fh = st.tile([P, SB], BF16)
            nc.vector.memset(fh[:], 0.0)
            h_t = []
            c_t = []
            for s in range(KH):
                hs = gp.tile([P, BC], BF16, name=f"h{s}", tag=f"h{s}")
                cs = gp.tile([P, BC], F32, name=f"c{s}", tag=f"c{s}")
                nc.vector.memset(hs[:], 0.0)
                nc.vector.memset(cs[:], 0.0)
                h_t.append(hs)
                c_t.append(cs)

            # ---- recurrence ----
            # PSUM banks: A_s holds gate chunks {i_s, f_s} (m = s, 8+s) over
            # steps [4g, 4g+4); B_s holds {j_s, o_s} (m = 4+s, 12+s) over
            # steps [4g+2, 4g+6) -- phase-offset so bank turnover (and the
            # N=256 x-part matmul bursts) spread across steps.
            SG = 4
            SGB = SG * BC
            assert t_steps % SG == 0 or t_steps == 2
            banks = {}  # (which, s) -> [tile, t0, tlen, mms_left]

            def xpart(ps, mi, m, t0, tlen):
                msl = slice(m * P, (m + 1) * P)
                m1sl = slice(G + m * P, G + (m + 1) * P)
                done = 0
                first = (mi == 0)
                while done < tlen:
                    ci, o = divmod((t0 + done) * BC, xcs * BC)
                    seg = min(tlen - done, xcs - (t0 + done) % xcs)
                    rsl = slice(o, o + seg * BC)
                    osl = ps[:, mi, done * BC:(done + seg) * BC]
                    nc.tensor.matmul(
                        osl, lhsT=wx01_sb[:, msl], rhs=xt0_c[ci][:, rsl],
                        start=first, stop=False, skip_group_check=True,
                    )
                    first = False
                    nc.tensor.matmul(
                        osl, lhsT=wx01_sb[:, m1sl], rhs=xt1_c[ci][:, rsl],
                        start=False, stop=False, skip_group_check=True,
                    )
                    nc.tensor.matmul(
                        osl, lhsT=wx2_sb[:, msl], rhs=xt2_c[ci][:, rsl],
                        start=False, stop=False, skip_group_check=True,
                    )
                    done += seg

            def open_bank(which, s, t0, tlen):
                ps = psum.tile([P, 2, SGB], F32, tag="zps",
                               name=f"ps{which}{s}_{t0}")
                ms = (s, 8 + s) if which == "A" else (4 + s, 12 + s)
                for mi, m in enumerate(ms):
                    xpart(ps, mi, m, t0, tlen)
                banks[(which, s)] = [ps, t0, tlen, 8 * tlen]

            for t in range(t_steps):
                if t % SG == 0:
                    for s in range(KH):
                        open_bank("A", s, t, min(SG, t_steps - t))
                if t == 0 or t % SG == 2:
                    t0 = t
                    tlen = 2 if t == 0 else min(SG, t_steps - t)
                    for s in range(KH):
                        open_bank("B", s, t0, tlen)

                # h-part: k-major waves across all banks
                for k in range(KH):
                    for s in range(KH):
                        for which in ("A", "B"):
                            ps, t0, tlen, left = banks[(which, s)]
                            ca = t - t0
                            ms = (s, 8 + s) if which == "A" else (4 + s, 12 + s)
                            for mi, m in enumerate(ms):
                                left -= 1
                                nc.tensor.matmul(
                                    ps[:, mi, ca * BC:(ca + 1) * BC],
                                    lhsT=wh_sb_k[k][:, m * P:(m + 1) * P],
                                    rhs=h_t[k][:],
                                    start=False, stop=(left == 0),
                                    skip_group_check=True,
                                )
                            banks[(which, s)][3] = left

                if t % MCH == 0 and t_steps >= MCH:
                    mchunk = mp.tile([P, MCH * SB], mybir.dt.uint8, tag="mchunk")
                    nc.sync.dma_start(mchunk[:], masks[t // MCH, :, :])
                elif t == 0:
                    mchunk = mp.tile([P, t_steps * SB], mybir.dt.uint8,
                                     name="mchunk_s", tag="mchunk")
                    nc.sync.dma_start(mchunk[:], masks[0, :, :])

                # gates + state update, pipelined per h-chunk s
                h_new = []
                c_new = []
                for s in range(KH):
                    psA, tA, _, _ = banks[("A", s)]
                    psB, tB, _, _ = banks[("B", s)]
                    ca, cb = t - tA, t - tB
                    sif = gp.tile([P, 2, BC], F32, name=f"sif{s}", tag=f"sif{s}")
                    tj = gp.tile([P, BC], F32, name=f"tj{s}", tag=f"tj{s}")
                    so = gp.tile([P, BC], F32, name=f"so{s}", tag=f"so{s}")
                    nc.scalar.activation(
                        sif[:], psA[:, :, ca * BC:(ca + 1) * BC], AF.Sigmoid
                    )
                    nc.scalar.activation(
                        tj[:], psB[:, 0, cb * BC:(cb + 1) * BC], AF.Tanh
                    )
                    nc.scalar.activation(
                        so[:], psB[:, 1, cb * BC:(cb + 1) * BC], AF.Sigmoid
                    )
                    ta = gp.tile([P, BC], F32, name=f"ta{s}", tag=f"ta{s}")
                    tb = gp.tile([P, BC], F32, name=f"tb{s}", tag=f"tb{s}")
                    cs = gp.tile([P, BC], F32, name=f"cn{s}", tag=f"c{s}")
                    tc_ = gp.tile([P, BC], F32, name=f"tc{s}", tag=f"tc{s}")
                    hs = gp.tile([P, BC], BF16, name=f"hn{s}", tag=f"h{s}")
                    nc.vector.tensor_mul(ta[:], c_t[s][:], sif[:, 1, :])
                    nc.vector.tensor_mul(tb[:], sif[:, 0, :], tj[:])
                    nc.vector.tensor_add(cs[:], ta[:], tb[:])
                    nc.scalar.activation(tc_[:], cs[:], AF.Tanh)
                    nc.vector.tensor_mul(hs[:], tc_[:], so[:])
                    tt = t % MCH if t_steps >= MCH else t
                    nc.vector.copy_predicated(
                        fh[:, s * BC:(s + 1) * BC],
                        mchunk[:, tt * SB + s * BC:tt * SB + (s + 1) * BC],
                        hs[:],
                    )
                    h_new.append(hs)
                    c_new.append(cs)
                h_t = h_new
                c_t = c_new

            # ---- MLP head ----
            # partial = (side rows of trans_w).T @ fh^T  -> [D, 64] transposed
            p_sb = st.tile([P, MD * BC], BF16)
            for m in range(MD):
                ps = psum.tile([P, BC], F32, tag="zps")
                for k in range(KH):
                    nc.tensor.matmul(
                        ps[:],
                        lhsT=twt_sb[:, k * D + m * P:k * D + (m + 1) * P],
                        rhs=fh[:, k * BC:(k + 1) * BC],
                        start=(k == 0), stop=(k == KH - 1),
                    )
                nc.vector.tensor_copy(p_sb[:, m * BC:(m + 1) * BC], ps[:])

            cin = dp.tile([P, MD * BC], BF16)
            cout = dp.tile([P, MD * BC], BF16)
            nc.sync.dma_start(cin[:], p_sb[:])
            nc.gpsimd.collective_compute(
                "AllReduce",
                mybir.AluOpType.add,
                replica_groups=[[0, 4], [1, 5], [2, 6], [3, 7]],
                ins=[cin.opt()],
                outs=[cout.opt()],
            )
            x1pre = st.tile([P, MD * BC], BF16)
            nc.sync.dma_start(x1pre[:], cout[:])
            xcur = st.tile([P, MD * BC], BF16)
            nc.scalar.activation(xcur[:], x1pre[:], AF.Relu)

            for layer in range(2):
                nxt = st.tile([P, MD * BC], BF16, tag=f"x{layer + 1}")
                out_f32 = None
                if layer == 1:
                    out_f32 = st.tile([P, MD * BC], F32, name="out_f32")
                for m in range(MD):
                    ps = psum.tile([P, BC], F32, tag="zps")
                    for k in range(MD):
                        off = (layer * MD + k) * D
                        nc.tensor.matmul(
                            ps[:],
                            lhsT=hww_sb[:, off + m * P:off + (m + 1) * P],
                            rhs=xcur[:, k * BC:(k + 1) * BC],
                            start=(k == 0), stop=(k == MD - 1),
                        )
                    if layer == 0:
                        nc.scalar.activation(
                            nxt[:, m * BC:(m + 1) * BC], ps[:], AF.Relu
                        )
                    else:
                        nc.scalar.activation(
                            out_f32[:, m * BC:(m + 1) * BC], ps[:], AF.Relu
                        )
                xcur = nxt
            nc.sync.dma_start(out_t[:], out_f32[:])

    nc.compile()
    return nc


_BUILD_CACHE: dict = {}


def _get_nc(t_steps: int = T):
    if t_steps not in _BUILD_CACHE:
        _BUILD_CACHE[t_steps] = _build(t_steps)
    return _BUILD_CACHE[t_steps]


def _core_inputs(embed, lengths, Wf, bf, trans_w, hidden_ws, side, t_steps):
    """Build the per-core input dict. embed [BC,T,E] f32, lengths [BC] i32."""
    bf16 = ml_dtypes.bfloat16
    TB = t_steps * BC

    # x transposed: xt[e, t, b]
    xt = np.ascontiguousarray(
        embed[:, :t_steps, :].transpose(2, 1, 0)
    ).astype(bf16)  # [E, t_steps, BC]
    xt0 = xt[0:P].reshape(P, TB)
    xt1 = xt[P:2 * P].reshape(P, TB)
    xt2 = np.empty((E2 + 1, TB), dtype=bf16)
    xt2[:E2] = xt[2 * P:E].reshape(E2, TB)
    xt2[E2] = np.ones(TB, dtype=bf16)

    Wb = Wf.astype(np.float32)
    wh = np.ascontiguousarray(
        Wb[E:].reshape(KH, P, G).transpose(1, 0, 2).reshape(P, KH * G)
    ).astype(bf16)
    wx01 = np.ascontiguousarray(
        Wb[0:2 * P].reshape(2, P, G).transpose(1, 0, 2).reshape(P, 2 * G)
    ).astype(bf16)
    badj = bf.astype(np.float32).copy()
    badj[2 * H:3 * H] += FORGET_BIAS
    wx2 = np.empty((E2 + 1, G), dtype=bf16)
    wx2[:E2] = Wb[2 * P:E].astype(bf16)
    wx2[E2] = badj.astype(bf16)

    # one-hot capture masks, replicated across partitions and h-chunks
    m_tb = np.zeros((t_steps, BC), dtype=np.uint8)
    cap = np.minimum(lengths.astype(np.int64), t_steps) - 1
    m_tb[cap, np.arange(BC)] = 1
    mch = min(MCH, t_steps)
    masks = np.ascontiguousarray(
        np.broadcast_to(
            m_tb[:, None, None, None, :], (t_steps, P, 1, KH, BC)
        ).reshape(t_steps // mch, mch, P, KH * BC).transpose(0, 2, 1, 3)
    ).reshape(t_steps // mch, P, mch * KH * BC)

    tw = trans_w[side * H:(side + 1) * H].astype(np.float32)
    twt = np.ascontiguousarray(
        tw.reshape(KH, P, D).transpose(1, 0, 2).reshape(P, KH * D)
    ).astype(bf16)
    hww = np.ascontiguousarray(
        hidden_ws.astype(np.float32).reshape(2, MD, P, D)
        .transpose(2, 0, 1, 3).reshape(P, 2 * MD * D)
    ).astype(bf16)

    return dict(xt0=xt0, xt1=xt1, xt2=xt2, wh=wh, wx01=wx01, wx2=wx2,
                masks=masks, twt=twt, hww=hww)


def prepare_in_maps(left_embed, right_embed, left_lengths, right_lengths,
                    W_left, b_left, W_right, b_right, trans_w, hidden_ws,
                    t_steps=T):
    in_maps = []
    for core in range(NCORES):
        side, q = divmod(core, 4)
        rows = slice(q * BC, (q + 1) * BC)
        if side == 0:
            emb, ln, Wf, bf = left_embed[rows], left_lengths[rows], W_left, b_left
        else:
            emb, ln, Wf, bf = right_embed[rows], right_lengths[rows], W_right, b_right
        in_maps.append(
            _core_inputs(np.asarray(emb), np.asarray(ln), np.asarray(Wf),
                         np.asarray(bf), np.asarray(trans_w),
                         np.asarray(hidden_ws), side, t_steps)
        )
    return in_maps


def _assemble(results):
    out = np.empty((B, D), dtype=np.float32)
    for q in range(4):
        # out_t [P, MD, BC] with out_t[p, m, b] = y[q*64+b, m*128+p]
        ot = results[q]["out_t"].reshape(P, MD, BC)
        out[q * BC:(q + 1) * BC] = ot.transpose(2, 1, 0).reshape(BC, D)
    return out


def kernel(left_embed, right_embed, left_lengths, right_lengths,
           W_left, b_left, W_right, b_right, trans_w, hidden_ws):
    nc = _get_nc(T)
    in_maps = prepare_in_maps(
        left_embed, right_embed, left_lengths, right_lengths,
        W_left, b_left, W_right, b_right, trans_w, hidden_ws, T
    )
    r = run_bass_kernel_spmd(nc, in_maps, list(range(NCORES)))
    return _assemble(r.results)


def run_traced(inputs, t_steps=T, **trace_kwargs):
    """test.py helper: run with NTFF tracing, return (output, BassKernelResults)."""
    nc = _get_nc(t_steps)
    in_maps = prepare_in_maps(t_steps=t_steps, **inputs)
    r = run_bass_kernel_spmd(
        nc, in_maps, list(range(NCORES)), trace=True, **trace_kwargs
    )
    return _assemble(r.results), r

